# revision 34
# baseline (speedup 1.0000x reference)
"""GTN (graph transformer network) forward on 8 Trainium2 cores.

Math (mirrors the reference; normalizations folded, matmuls re-associated):
  A[t]  = dense adjacency from edge lists              (host, bincount)
  A1 = softmax(w_l0_c1) . A ; A2 = softmax(w_l0_c2) . A ; A3 = softmax(w_l1_c1) . A
  U  = A1 @ A2 @ A3  (never materialized!)
  The output only needs U @ XW (XW = X @ gcn_w, [N,128]) and rowsum(U):
    U @ XW     = A1 @ (A2 @ (A3 @ XW))      three [N,N]@[N,128] products
    rowsum(U)  = A1 @ (A2 @ rowsum(A3))     two GEMVs, done on host
  and only at the unique target_x rows (~900 of 4096).  This is ~25x fewer
  FLOPs than forming A1@A2@A3.  Row-normalizing only at the end is exact:
  row scaling commutes through matmul and all entries are >= 0.
  y = relu(Z/rowsum + b) -> channel concat -> target gather -> linear (host).

Sharding: 2 channels x 4-core groups, with NO mid-kernel gathers.  Core r
of channel c computes, entirely locally (contraction sharding):
  Y3_r = A3_c[rows_r] @ XW                   rows_r = r*1024 ... +1024
  P2_r = A2_c[:, rows_r] @ Y3_r              partial, all 4096 rows
  Zp_r = A1_c[tgt] @ P2_r                    partial, all padded target rows
then ReduceScatter(add) over the channel group sums the partials and
scatters target rows back:  Z = sum_r Zp_r  (linearity).  Stage 1 runs in
two halves: half 0 is interleaved into stage 2 (folding each fresh batch
of P2 chunks into 4 persistent PSUM accumulators) so its ReduceScatter
fires at stage-2 completion, overlapping half 1's slab stream + matmuls.
Y3 and P2 never leave SBUF.  Device inputs are bf16 slabs of the transposed
combos (host builds A^T for free by swapping src/dst in the bincount);
slab DMAs stream on one hardware queue in consumption order and the
matmuls ride the stream (piece-granular subtile dependencies).  A tiny
warm-up collective at t~0 absorbs the ~65us cold-start of the collective
firmware while the slabs stream.
"""

import os
import time
import numpy as np
from contextlib import ExitStack

NUM_EDGE = 5
C = 2
N = 4096
W_IN = 512
W_OUT = 128
NCORES = 8
P = 128
NGRP = 4                    # cores per channel group
RLOC = N // NGRP            # 1024 rows per core
NK = N // P                 # 32 contraction chunks (full N)
KL = RLOC // P              # 8 local contraction chunks (stage 2)
NM = RLOC // P              # 8 output row tiles (stages 3, 1)
NMF = N // P                # 32 output row tiles (stage 2)
NTGT_TOT = 1024             # padded unique-target rows per channel
NTGT = NTGT_TOT // NGRP     # 256 target rows landing on each core
DOUT = W_OUT                # 128
NPIECE = 8                  # DMA pieces per column slab
KPP = NK // NPIECE          # k-chunks per piece
NPC2 = 4                    # column pieces for the stage-2 row slab
MW2 = NMF // NPC2           # m-tiles per stage-2 piece
GROUPS = [[0, 1, 2, 3], [4, 5, 6, 7]]

_NC_CACHE = {}
LAST_EXEC_NS = None
LAST_RESULTS = None


def _build_nc():
    import concourse.tile as tile
    from concourse import bacc, mybir

    nc = bacc.Bacc("TRN2", target_bir_lowering=False, debug=False,
                   num_devices=NCORES)
    f32 = mybir.dt.float32
    bf16 = mybir.dt.bfloat16

    # l3[j, i] = A3^T[c][j, rows_r[i]]      (column slab, [N, RLOC])
    # l2[i, j] = A2^T[c][rows_r[i], j]      (row slab,    [RLOC, N])
    # l1[j, t] = A1^T[c][j, tgt_pad[t]]     (column slab, [N, NTGT_TOT])
    l3 = nc.dram_tensor("l3", [N, RLOC], bf16, kind="ExternalInput").ap()
    l2 = nc.dram_tensor("l2", [RLOC, N], bf16, kind="ExternalInput").ap()
    l1 = nc.dram_tensor("l1", [N, NTGT_TOT], bf16, kind="ExternalInput").ap()
    # xw prechunked on host: xw[p, k*DOUT+d] = XW[P*k+p, d]
    xw = nc.dram_tensor("xw", [P, NK * DOUT], bf16, kind="ExternalInput").ap()
    z = nc.dram_tensor("z", [NTGT, DOUT], bf16, kind="ExternalOutput").ap()

    with tile.TileContext(nc) as tc, ExitStack() as ctx:
        xwp = ctx.enter_context(tc.tile_pool(name="xwp", bufs=1))
        slabp = ctx.enter_context(tc.tile_pool(name="slabp", bufs=2))
        ysbp = ctx.enter_context(tc.tile_pool(name="ysbp", bufs=1))
        outp = ctx.enter_context(tc.tile_pool(name="outp", bufs=2))
        psp = ctx.enter_context(tc.tile_pool(name="psp", bufs=4, space="PSUM"))
        dramp = ctx.enter_context(tc.tile_pool(name="dramp", bufs=1,
                                               space="DRAM"))

        xw_sb = xwp.tile([P, NK * DOUT], bf16, tag="xw")
        nc.gpsimd.dma_start(xw_sb[:], xw)

        # tiny warm-up collective: completes during the slab stream so the
        # CC pipeline is hot when the real ReduceScatter arrives
        warm_in = dramp.tile([NGRP, 64], bf16, tag="warm_in")
        warm_out = dramp.tile([1, 64], bf16, tag="warm_out")
        nc.gpsimd.dma_start(warm_in[:], xw[0:NGRP, 0:64])
        nc.gpsimd.collective_compute(
            "ReduceScatter", mybir.AluOpType.add,
            replica_groups=GROUPS,
            ins=[warm_in.opt()], outs=[warm_out.opt()])

        def load_colslab(slab_dram, width, kbounds, tag="slab", bufs=None):
            # sb[p, k*width + i] = slab[P*k + p, i]; 2KB contiguous runs,
            # pieces (k-ranges given by kbounds) so downstream matmuls
            # start on piece 0.  All bulk loads share the scalar engine's
            # hardware queue: in-queue ordering streams them in
            # consumption order.
            sb = slabp.tile([P, NK * width], bf16, tag=tag, bufs=bufs)
            for k0, k1 in zip(kbounds, kbounds[1:]):
                nc.scalar.dma_start(
                    sb[:, k0 * width:k1 * width]
                      .rearrange("p (k i) -> p k i", k=k1 - k0),
                    slab_dram[k0 * P:k1 * P, :]
                      .rearrange("(k p) i -> p k i", p=P))
            return sb

        def col_stage(slab_sb, width, mlist, rhs_sb, out_sb, sname,
                      kbounds):
            # out[m*P+p, d] = sum_k slab[k, m*P+p] * rhs[k, d], m in mlist
            # piece-major so matmuls ride the slab DMA stream; m-outer
            # within a piece keeps consecutive matmuls on one PSUM bank;
            # last piece's copies overlap the remaining matmuls
            accs = [psp.tile([P, DOUT], f32, tag="acc",
                             name=f"acc_{sname}_{m}") for m in mlist]
            npiece = len(kbounds) - 1
            for pc in range(npiece):
                last = pc == npiece - 1
                for i, m in enumerate(mlist):
                    for k in range(kbounds[pc], kbounds[pc + 1]):
                        nc.tensor.matmul(
                            accs[i][:],
                            slab_sb[:, k * width + m * P:
                                    k * width + (m + 1) * P],
                            rhs_sb[:, k * DOUT:(k + 1) * DOUT],
                            start=(k == 0), stop=(last and k == NK - 1),
                            skip_group_check=True)
                    if last:
                        nc.vector.tensor_copy(
                            out_sb[:, m * DOUT:(m + 1) * DOUT], accs[i][:])

        NSPLIT = 2
        HTGT = NTGT_TOT // NSPLIT     # 512 target rows per split
        HM = HTGT // P                # 4 m-tiles per split
        KB1 = [0, 4, 8, 12, 16, 20, 24, 28, 32]

        def reduce_scatter(zsb, h):
            zp = dramp.tile([HTGT, DOUT], bf16, tag=f"zp{h}",
                            name=f"zp_{h}")
            zrs = dramp.tile([HTGT // NGRP, DOUT], bf16, tag=f"zrs{h}",
                             name=f"zrs_{h}")
            nc.gpsimd.dma_start(
                zp.rearrange("(m p) d -> p m d", p=P),
                zsb[:].rearrange("p (m d) -> p m d", m=HM))
            nc.gpsimd.collective_compute(
                "ReduceScatter", mybir.AluOpType.add,
                replica_groups=GROUPS,
                ins=[zp.opt()], outs=[zrs.opt()])
            # core r receives rows [64r,64r+64) of split h
            w = NTGT // NSPLIT
            nc.gpsimd.dma_start(z[h * w:(h + 1) * w, :], zrs[:])

        # ---- stage 3: Y3_r = A3_c[rows_r] @ XW  (stays in SBUF) ----
        # two 4-accumulator passes (tag "acc" holds 4 PSUM banks; the other
        # 4 banks belong to the interleaved stage-1 half below); first
        # piece halved so the first matmul fires earlier
        KB3 = [0, 2, 4, 8, 12, 16, 20, 24, 28, 32]
        sb3 = load_colslab(l3, RLOC, KB3)
        y3sb = ysbp.tile([P, KL * DOUT], bf16, tag="y3")
        col_stage(sb3, RLOC, [0, 1, 2, 3], xw_sb, y3sb, "s3a", KB3)
        col_stage(sb3, RLOC, [4, 5, 6, 7], xw_sb, y3sb, "s3b",
                  [0, NK])

        # ---- stage 1 half 0 slab streams before the stage-2 slab so its
        # matmuls can interleave with stage 2 ----
        sb1h0 = load_colslab(l1[:, 0:HTGT], HTGT, KB1)

        # ---- stage 2 (P2_r = A2_c[:, rows_r] @ Y3_r) interleaved with
        # stage-1 half 0 (Zp_r[0:512] = A1_c[tgt 0:512] @ P2_r): after each
        # stage-2 column piece lands its 8 P2 chunks, the four persistent
        # half-0 accumulators fold those chunks in, so the first
        # ReduceScatter fires right at stage-2 completion ----
        sb2 = slabp.tile([P, KL * N], bf16, tag="slab")
        cw = MW2 * P
        for pc in range(NPC2):
            nc.scalar.dma_start(
                sb2[:].rearrange("p (kl j) -> p kl j", kl=KL)
                      [:, :, pc * cw:(pc + 1) * cw],
                l2.rearrange("(kl p) j -> p kl j", p=P)
                  [:, :, pc * cw:(pc + 1) * cw])
        p2sb = ysbp.tile([P, NMF * DOUT], bf16, tag="p2")
        acc1 = [psp.tile([P, DOUT], f32, tag="acc1", name=f"acc1_{m}")
                for m in range(HM)]
        zsb0 = outp.tile([P, HM * DOUT], bf16, tag="zout", name="zsb_0")
        for pc in range(NPC2):
            for mm in range(MW2):
                m = pc * MW2 + mm
                acc = psp.tile([P, DOUT], f32, tag="acc", name=f"acc_s2_{m}")
                for kl in range(KL):
                    nc.tensor.matmul(
                        acc[:],
                        sb2[:, kl * N + m * P: kl * N + (m + 1) * P],
                        y3sb[:, kl * DOUT:(kl + 1) * DOUT],
                        start=(kl == 0), stop=(kl == KL - 1),
                        skip_group_check=True)
                nc.vector.tensor_copy(
                    p2sb[:, m * DOUT:(m + 1) * DOUT], acc[:])
            last = pc == NPC2 - 1
            for i in range(HM):
                for k in range(pc * MW2, (pc + 1) * MW2):
                    nc.tensor.matmul(
                        acc1[i][:],
                        sb1h0[:, k * HTGT + i * P: k * HTGT + (i + 1) * P],
                        p2sb[:, k * DOUT:(k + 1) * DOUT],
                        start=(k == 0), stop=(last and k == NK - 1),
                        skip_group_check=True)
                if last:
                    nc.vector.tensor_copy(
                        zsb0[:, i * DOUT:(i + 1) * DOUT], acc1[i][:])
        reduce_scatter(zsb0, 0)

        # ---- stage 1 half 1, then its ReduceScatter ----
        sb1h1 = load_colslab(l1[:, HTGT:NTGT_TOT], HTGT, KB1)
        zsb1 = outp.tile([P, HM * DOUT], bf16, tag="zout", name="zsb_1")
        col_stage(sb1h1, HTGT, [0, 1, 2, 3], p2sb, zsb1, "s1h1", KB1)
        reduce_scatter(zsb1, 1)

    nc.compile()
    return nc


def _get_nc():
    if "nc" not in _NC_CACHE:
        _NC_CACHE["nc"] = _build_nc()
    return _NC_CACHE["nc"]


def _softmax_rows(w):
    w = np.asarray(w, np.float32)
    e = np.exp(w - w.max(axis=1, keepdims=True))
    return (e / e.sum(axis=1, keepdims=True)).astype(np.float32)


def _install_ntff_hook():
    """Recreate antenv.axon_hooks if the image lacks it (profiling only)."""
    import sys
    import types
    try:
        from antenv.axon_hooks import get_axon_ntff_profile_hook  # noqa: F401
        return
    except ImportError:
        pass
    try:
        from trn_agent_boot.trn_boot import _ntff_profile_via_ctypes
        import antenv
        mod = types.ModuleType("antenv.axon_hooks")
        state = {"h": None}
        mod.set_axon_ntff_profile_hook = lambda h: state.__setitem__("h", h)
        mod.get_axon_ntff_profile_hook = lambda: state["h"]
        sys.modules["antenv.axon_hooks"] = mod
        antenv.axon_hooks = mod
        mod.set_axon_ntff_profile_hook(
            _ntff_profile_via_ctypes("/opt/axon/libaxon_pjrt.so"))
    except Exception:
        pass


def kernel(edge_index, edge_value, X, target_x, w_l0_c1, w_l0_c2, w_l1_c1,
           gcn_w, gcn_b, lin_w, lin_b):
    global LAST_EXEC_NS, LAST_RESULTS
    import ml_dtypes
    from concourse.bass_utils import run_bass_kernel_spmd

    bf16 = ml_dtypes.bfloat16

    # transposed dense adjacency stack [NUM_EDGE, N*N] (dst-major == A^T),
    # duplicate edges summed
    src = np.asarray(edge_index[:, 0], np.int64)
    dst = np.asarray(edge_index[:, 1], np.int64)
    ATf = np.empty((NUM_EDGE, N * N), np.float32)
    for t in range(NUM_EDGE):
        flat = dst[t] * N + src[t]
        ATf[t] = np.bincount(flat, weights=np.asarray(edge_value[t], np.float64),
                             minlength=N * N).astype(np.float32)

    def combo(w):
        f = _softmax_rows(w)                 # [C, NUM_EDGE]
        return (f @ ATf).reshape(C, N, N)    # transposed combos [C, N, N]

    A1T = combo(w_l0_c1)
    A2T = combo(w_l0_c2)
    A3T = combo(w_l1_c1)
    ATf = None  # free

    # rowsum(U) = A1 @ (A2 @ rowsum(A3)), as cheap host GEMVs on the
    # transposed combos: A @ v == v @ A^T.
    s = np.empty((C, N), np.float32)
    for c in range(C):
        v = A3T[c].sum(axis=0)               # rowsum(A3_c)
        s[c] = (v @ A2T[c]) @ A1T[c]

    XW = np.asarray(X, np.float32) @ np.asarray(gcn_w, np.float32)  # [N, 128]
    # prechunk to the SBUF layout: xwb[p, k*DOUT+d] = XW[P*k+p, d]
    xwb = np.ascontiguousarray(
        XW.astype(bf16).reshape(NK, P, DOUT).transpose(1, 0, 2)
        .reshape(P, NK * DOUT))

    # unique target rows, zero-padded to NTGT_TOT per channel
    tgt = np.asarray(target_x, np.int64)
    u, inv = np.unique(tgt, return_inverse=True)
    nu = len(u)
    assert nu <= NTGT_TOT, nu

    A1Tb = A1T.astype(bf16)
    A2Tb = A2T.astype(bf16)
    A3Tb = A3T.astype(bf16)
    A1T = A2T = A3T = None

    # l1 is identical across a channel group (stage 1 is contraction-
    # sharded): [N, NTGT_TOT] with zero columns past nu
    l1_by_c = []
    for c in range(C):
        l1c = np.zeros((N, NTGT_TOT), bf16)
        l1c[:, :nu] = A1Tb[c][:, u]
        l1_by_c.append(l1c)

    in_maps = []
    for ci in range(NCORES):
        c, r = divmod(ci, NGRP)
        sl = slice(r * RLOC, (r + 1) * RLOC)
        in_maps.append({
            "l1": l1_by_c[c],
            "l2": A2Tb[c][sl, :],                       # row slab, contiguous
            "l3": np.ascontiguousarray(A3Tb[c][:, sl]),  # column slab
            "xw": xwb,
        })

    nc = _get_nc()
    _install_ntff_hook()
    trace = os.environ.get("GTN_TRACE", "1") != "0"
    t0 = time.time()
    res = None
    if trace:
        try:
            res = run_bass_kernel_spmd(nc, in_maps, list(range(NCORES)),
                                       trace=True,
                                       trace_cores=list(range(NCORES)))
        except Exception as e:
            import traceback
            traceback.print_exc()
            print(f"[kernel] trace run failed ({e!r}); retrying untraced")
            res = None
    if res is None:
        res = run_bass_kernel_spmd(nc, in_maps, list(range(NCORES)),
                                   trace=False)
    wall_ns = int((time.time() - t0) * 1e9)
    LAST_EXEC_NS = res.exec_time_ns if res.exec_time_ns else wall_ns
    LAST_RESULTS = res

    # core c*NGRP+r returns z[w*h : w*(h+1)] = rows [QT*h + w*r, +w) of the
    # padded target list, for each of the NSPLIT stage-1 splits
    NSPLIT = 2
    QT = NTGT_TOT // NSPLIT       # 256 padded target rows per split
    w = NTGT // NSPLIT            # 64 rows per (split, core)
    Zu = np.empty((C, nu, DOUT), np.float32)
    Zpad = np.empty((NTGT_TOT, DOUT), np.float32)
    for c in range(C):
        for r in range(NGRP):
            zc = np.asarray(res.results[c * NGRP + r]["z"], np.float32)
            for h in range(NSPLIT):
                Zpad[QT * h + w * r: QT * h + w * (r + 1)] = \
                    zc[w * h: w * (h + 1)]
        Zu[c] = Zpad[:nu]
    su = s[:, u]                                             # [C, nu]
    with np.errstate(divide="ignore", invalid="ignore"):
        sinv = np.where(su == 0, 0.0, 1.0 / su).astype(np.float32)
    Hn = Zu * sinv[:, :, None]                               # [C, nu, 128]
    Xc = np.maximum(Hn + np.asarray(gcn_b, np.float32)[None, None, :], 0.0)
    X_ = Xc.transpose(1, 0, 2).reshape(nu, C * W_OUT)        # [nu, 256]
    y = X_[inv] @ np.asarray(lin_w, np.float32)
    y = y + np.asarray(lin_b, np.float32)
    return y.astype(np.float32)


# revision 38
# speedup vs baseline: 1.0001x; 1.0001x over previous
"""GTN (graph transformer network) forward on 8 Trainium2 cores.

Math (mirrors the reference; normalizations folded, matmuls re-associated):
  A[t]  = dense adjacency from edge lists              (host, bincount)
  A1 = softmax(w_l0_c1) . A ; A2 = softmax(w_l0_c2) . A ; A3 = softmax(w_l1_c1) . A
  U  = A1 @ A2 @ A3  (never materialized!)
  The output only needs U @ XW (XW = X @ gcn_w, [N,128]) and rowsum(U):
    U @ XW     = A1 @ (A2 @ (A3 @ XW))      three [N,N]@[N,128] products
    rowsum(U)  = A1 @ (A2 @ rowsum(A3))     two GEMVs, done on host
  and only at the unique target_x rows (~900 of 4096).  This is ~25x fewer
  FLOPs than forming A1@A2@A3.  Row-normalizing only at the end is exact:
  row scaling commutes through matmul and all entries are >= 0.
  y = relu(Z/rowsum + b) -> channel concat -> target gather -> linear (host).

Sharding: 2 channels x 4-core groups, with NO mid-kernel gathers.  Core r
of channel c computes, entirely locally (contraction sharding):
  Y3_r = A3_c[rows_r] @ XW                   rows_r = r*1024 ... +1024
  P2_r = A2_c[:, rows_r] @ Y3_r              partial, all 4096 rows
  Zp_r = A1_c[tgt] @ P2_r                    partial, all padded target rows
then ReduceScatter(add) over the channel group sums the partials and
scatters target rows back:  Z = sum_r Zp_r  (linearity).  Stage 1 runs in
two halves: half 0 is interleaved into stage 2 (folding each fresh batch
of P2 chunks into 4 persistent PSUM accumulators) so its ReduceScatter
fires at stage-2 completion, overlapping half 1's slab stream + matmuls.
Y3 and P2 never leave SBUF.  Device inputs are bf16 slabs of the transposed
combos (host builds A^T for free by swapping src/dst in the bincount);
slab DMAs stream on one hardware queue in consumption order and the
matmuls ride the stream (piece-granular subtile dependencies).  A tiny
warm-up collective at t~0 absorbs the ~65us cold-start of the collective
firmware while the slabs stream.
"""

import os
import time
import numpy as np
from contextlib import ExitStack

NUM_EDGE = 5
C = 2
N = 4096
W_IN = 512
W_OUT = 128
NCORES = 8
P = 128
NGRP = 4                    # cores per channel group
RLOC = N // NGRP            # 1024 rows per core
NK = N // P                 # 32 contraction chunks (full N)
KL = RLOC // P              # 8 local contraction chunks (stage 2)
NM = RLOC // P              # 8 output row tiles (stages 3, 1)
NMF = N // P                # 32 output row tiles (stage 2)
NTGT_TOT = 1024             # padded unique-target rows per channel
NTGT = NTGT_TOT // NGRP     # 256 target rows landing on each core
DOUT = W_OUT                # 128
NPIECE = 8                  # DMA pieces per column slab
KPP = NK // NPIECE          # k-chunks per piece
NPC2 = 4                    # column pieces for the stage-2 row slab
MW2 = NMF // NPC2           # m-tiles per stage-2 piece
GROUPS = [[0, 1, 2, 3], [4, 5, 6, 7]]

_NC_CACHE = {}
LAST_EXEC_NS = None
LAST_RESULTS = None


def _build_nc():
    import concourse.tile as tile
    from concourse import bacc, mybir

    nc = bacc.Bacc("TRN2", target_bir_lowering=False, debug=False,
                   num_devices=NCORES)
    f32 = mybir.dt.float32
    bf16 = mybir.dt.bfloat16

    # l3[j, i] = A3^T[c][j, rows_r[i]]      (column slab, [N, RLOC])
    # l2[i, j] = A2^T[c][rows_r[i], j]      (row slab,    [RLOC, N])
    # l1[j, t] = A1^T[c][j, tgt_pad[t]]     (column slab, [N, NTGT_TOT])
    l3 = nc.dram_tensor("l3", [N, RLOC], bf16, kind="ExternalInput").ap()
    l2 = nc.dram_tensor("l2", [RLOC, N], bf16, kind="ExternalInput").ap()
    l1 = nc.dram_tensor("l1", [N, NTGT_TOT], bf16, kind="ExternalInput").ap()
    # xw prechunked on host: xw[p, k*DOUT+d] = XW[P*k+p, d]
    xw = nc.dram_tensor("xw", [P, NK * DOUT], bf16, kind="ExternalInput").ap()
    # z keeps the SBUF layout: z[h, i, m*DOUT+d] = sum_r Zp[tgt m*P+32r+i, d]
    # of split h, where r = this core's rank (host un-permutes)
    z = nc.dram_tensor("z", [2, P // NGRP, (NTGT_TOT // 2 // P) * DOUT],
                       bf16, kind="ExternalOutput").ap()

    with tile.TileContext(nc) as tc, ExitStack() as ctx:
        xwp = ctx.enter_context(tc.tile_pool(name="xwp", bufs=1))
        slabp = ctx.enter_context(tc.tile_pool(name="slabp", bufs=2))
        ysbp = ctx.enter_context(tc.tile_pool(name="ysbp", bufs=1))
        outp = ctx.enter_context(tc.tile_pool(name="outp", bufs=2))
        psp = ctx.enter_context(tc.tile_pool(name="psp", bufs=4, space="PSUM"))
        dramp = ctx.enter_context(tc.tile_pool(name="dramp", bufs=1,
                                               space="DRAM"))

        # first few chunks in their own DMA so the first matmul isn't
        # gated by the full 1MB load
        xw_sb = xwp.tile([P, NK * DOUT], bf16, tag="xw")
        nc.gpsimd.dma_start(xw_sb[:, 0:2 * DOUT], xw[:, 0:2 * DOUT])
        nc.gpsimd.dma_start(xw_sb[:, 2 * DOUT:], xw[:, 2 * DOUT:])

        # tiny warm-up collective: completes during the slab stream so the
        # CC pipeline is hot when the real ReduceScatter arrives
        warm_in = dramp.tile([NGRP, 64], bf16, tag="warm_in")
        warm_out = dramp.tile([1, 64], bf16, tag="warm_out")
        nc.gpsimd.dma_start(warm_in[:], xw[0:NGRP, 0:64])
        nc.gpsimd.collective_compute(
            "ReduceScatter", mybir.AluOpType.add,
            replica_groups=GROUPS,
            ins=[warm_in.opt()], outs=[warm_out.opt()])

        def load_colslab(slab_dram, width, kbounds, tag="slab", bufs=None):
            # sb[p, k*width + i] = slab[P*k + p, i]; 2KB contiguous runs,
            # pieces (k-ranges given by kbounds) so downstream matmuls
            # start on piece 0.  All bulk loads share the scalar engine's
            # hardware queue: in-queue ordering streams them in
            # consumption order.
            sb = slabp.tile([P, NK * width], bf16, tag=tag, bufs=bufs)
            for k0, k1 in zip(kbounds, kbounds[1:]):
                nc.scalar.dma_start(
                    sb[:, k0 * width:k1 * width]
                      .rearrange("p (k i) -> p k i", k=k1 - k0),
                    slab_dram[k0 * P:k1 * P, :]
                      .rearrange("(k p) i -> p k i", p=P))
            return sb

        def col_stage(slab_sb, width, mlist, rhs_sb, out_sb, sname,
                      kbounds):
            # out[m*P+p, d] = sum_k slab[k, m*P+p] * rhs[k, d], m in mlist
            # piece-major so matmuls ride the slab DMA stream; m-outer
            # within a piece keeps consecutive matmuls on one PSUM bank;
            # last piece's copies overlap the remaining matmuls
            accs = [psp.tile([P, DOUT], f32, tag="acc",
                             name=f"acc_{sname}_{m}") for m in mlist]
            npiece = len(kbounds) - 1
            for pc in range(npiece):
                last = pc == npiece - 1
                for i, m in enumerate(mlist):
                    for k in range(kbounds[pc], kbounds[pc + 1]):
                        nc.tensor.matmul(
                            accs[i][:],
                            slab_sb[:, k * width + m * P:
                                    k * width + (m + 1) * P],
                            rhs_sb[:, k * DOUT:(k + 1) * DOUT],
                            start=(k == 0), stop=(last and k == NK - 1),
                            skip_group_check=True)
                    if last:
                        nc.vector.tensor_copy(
                            out_sb[:, m * DOUT:(m + 1) * DOUT], accs[i][:])

        NSPLIT = 2
        HTGT = NTGT_TOT // NSPLIT     # 512 target rows per split
        HM = HTGT // P                # 4 m-tiles per split
        KB1 = [0, 4, 8, 12, 16, 20, 24, 28, 32]

        def reduce_scatter(zsb, h):
            # zp keeps zsb's SBUF layout (contiguous dump; the rearranged
            # write was 256B-run-bound and cost ~7us).  ReduceScatter sums
            # flat buffers elementwise, so layout is free to choose; core r
            # receives partitions [32r, 32r+32) -- host un-permutes.
            zp = dramp.tile([P, HM * DOUT], bf16, tag=f"zp{h}",
                            name=f"zp_{h}")
            zrs = dramp.tile([P // NGRP, HM * DOUT], bf16, tag=f"zrs{h}",
                             name=f"zrs_{h}")
            nc.gpsimd.dma_start(zp[:], zsb[:])
            nc.gpsimd.collective_compute(
                "ReduceScatter", mybir.AluOpType.add,
                replica_groups=GROUPS,
                ins=[zp.opt()], outs=[zrs.opt()])
            nc.gpsimd.dma_start(z[h], zrs[:])

        # ---- stage 3: Y3_r = A3_c[rows_r] @ XW  (stays in SBUF) ----
        # two 4-accumulator passes (tag "acc" holds 4 PSUM banks; the other
        # 4 banks belong to the interleaved stage-1 half below); first
        # piece halved so the first matmul fires earlier
        KB3 = [0, 2, 4, 8, 12, 16, 20, 24, 28, 32]
        sb3 = load_colslab(l3, RLOC, KB3)
        y3sb = ysbp.tile([P, KL * DOUT], bf16, tag="y3")
        col_stage(sb3, RLOC, [0, 1, 2, 3], xw_sb, y3sb, "s3a", KB3)
        col_stage(sb3, RLOC, [4, 5, 6, 7], xw_sb, y3sb, "s3b",
                  [0, NK])

        # ---- stage 1 half 0 slab streams before the stage-2 slab so its
        # matmuls can interleave with stage 2 ----
        sb1h0 = load_colslab(l1[:, 0:HTGT], HTGT, KB1)

        # ---- stage 2 (P2_r = A2_c[:, rows_r] @ Y3_r) interleaved with
        # stage-1 half 0 (Zp_r[0:512] = A1_c[tgt 0:512] @ P2_r): after each
        # stage-2 column piece lands its 8 P2 chunks, the four persistent
        # half-0 accumulators fold those chunks in, so the first
        # ReduceScatter fires right at stage-2 completion ----
        sb2 = slabp.tile([P, KL * N], bf16, tag="slab")
        cw = MW2 * P
        for pc in range(NPC2):
            nc.scalar.dma_start(
                sb2[:].rearrange("p (kl j) -> p kl j", kl=KL)
                      [:, :, pc * cw:(pc + 1) * cw],
                l2.rearrange("(kl p) j -> p kl j", p=P)
                  [:, :, pc * cw:(pc + 1) * cw])
        p2sb = ysbp.tile([P, NMF * DOUT], bf16, tag="p2")
        acc1 = [psp.tile([P, DOUT], f32, tag="acc1", name=f"acc1_{m}")
                for m in range(HM)]
        zsb0 = outp.tile([P, HM * DOUT], bf16, tag="zout", name="zsb_0")
        for pc in range(NPC2):
            for mm in range(MW2):
                m = pc * MW2 + mm
                acc = psp.tile([P, DOUT], f32, tag="acc", name=f"acc_s2_{m}")
                for kl in range(KL):
                    nc.tensor.matmul(
                        acc[:],
                        sb2[:, kl * N + m * P: kl * N + (m + 1) * P],
                        y3sb[:, kl * DOUT:(kl + 1) * DOUT],
                        start=(kl == 0), stop=(kl == KL - 1),
                        skip_group_check=True)
                nc.vector.tensor_copy(
                    p2sb[:, m * DOUT:(m + 1) * DOUT], acc[:])
            last = pc == NPC2 - 1
            for i in range(HM):
                for k in range(pc * MW2, (pc + 1) * MW2):
                    nc.tensor.matmul(
                        acc1[i][:],
                        sb1h0[:, k * HTGT + i * P: k * HTGT + (i + 1) * P],
                        p2sb[:, k * DOUT:(k + 1) * DOUT],
                        start=(k == 0), stop=(last and k == NK - 1),
                        skip_group_check=True)
                if last:
                    nc.vector.tensor_copy(
                        zsb0[:, i * DOUT:(i + 1) * DOUT], acc1[i][:])
        reduce_scatter(zsb0, 0)

        # ---- stage 1 half 1, then its ReduceScatter ----
        sb1h1 = load_colslab(l1[:, HTGT:NTGT_TOT], HTGT, KB1)
        zsb1 = outp.tile([P, HM * DOUT], bf16, tag="zout", name="zsb_1")
        col_stage(sb1h1, HTGT, [0, 1, 2, 3], p2sb, zsb1, "s1h1", KB1)
        reduce_scatter(zsb1, 1)

    nc.compile()
    return nc


def _get_nc():
    if "nc" not in _NC_CACHE:
        _NC_CACHE["nc"] = _build_nc()
    return _NC_CACHE["nc"]


def _softmax_rows(w):
    w = np.asarray(w, np.float32)
    e = np.exp(w - w.max(axis=1, keepdims=True))
    return (e / e.sum(axis=1, keepdims=True)).astype(np.float32)


def _install_ntff_hook():
    """Recreate antenv.axon_hooks if the image lacks it (profiling only)."""
    import sys
    import types
    try:
        from antenv.axon_hooks import get_axon_ntff_profile_hook  # noqa: F401
        return
    except ImportError:
        pass
    try:
        from trn_agent_boot.trn_boot import _ntff_profile_via_ctypes
        import antenv
        mod = types.ModuleType("antenv.axon_hooks")
        state = {"h": None}
        mod.set_axon_ntff_profile_hook = lambda h: state.__setitem__("h", h)
        mod.get_axon_ntff_profile_hook = lambda: state["h"]
        sys.modules["antenv.axon_hooks"] = mod
        antenv.axon_hooks = mod
        mod.set_axon_ntff_profile_hook(
            _ntff_profile_via_ctypes("/opt/axon/libaxon_pjrt.so"))
    except Exception:
        pass


def kernel(edge_index, edge_value, X, target_x, w_l0_c1, w_l0_c2, w_l1_c1,
           gcn_w, gcn_b, lin_w, lin_b):
    global LAST_EXEC_NS, LAST_RESULTS
    import ml_dtypes
    from concourse.bass_utils import run_bass_kernel_spmd

    bf16 = ml_dtypes.bfloat16

    # transposed dense adjacency stack [NUM_EDGE, N*N] (dst-major == A^T),
    # duplicate edges summed
    src = np.asarray(edge_index[:, 0], np.int64)
    dst = np.asarray(edge_index[:, 1], np.int64)
    ATf = np.empty((NUM_EDGE, N * N), np.float32)
    for t in range(NUM_EDGE):
        flat = dst[t] * N + src[t]
        ATf[t] = np.bincount(flat, weights=np.asarray(edge_value[t], np.float64),
                             minlength=N * N).astype(np.float32)

    def combo(w):
        f = _softmax_rows(w)                 # [C, NUM_EDGE]
        return (f @ ATf).reshape(C, N, N)    # transposed combos [C, N, N]

    A1T = combo(w_l0_c1)
    A2T = combo(w_l0_c2)
    A3T = combo(w_l1_c1)
    ATf = None  # free

    # rowsum(U) = A1 @ (A2 @ rowsum(A3)), as cheap host GEMVs on the
    # transposed combos: A @ v == v @ A^T.
    s = np.empty((C, N), np.float32)
    for c in range(C):
        v = A3T[c].sum(axis=0)               # rowsum(A3_c)
        s[c] = (v @ A2T[c]) @ A1T[c]

    XW = np.asarray(X, np.float32) @ np.asarray(gcn_w, np.float32)  # [N, 128]
    # prechunk to the SBUF layout: xwb[p, k*DOUT+d] = XW[P*k+p, d]
    xwb = np.ascontiguousarray(
        XW.astype(bf16).reshape(NK, P, DOUT).transpose(1, 0, 2)
        .reshape(P, NK * DOUT))

    # unique target rows, zero-padded to NTGT_TOT per channel
    tgt = np.asarray(target_x, np.int64)
    u, inv = np.unique(tgt, return_inverse=True)
    nu = len(u)
    assert nu <= NTGT_TOT, nu

    A1Tb = A1T.astype(bf16)
    A2Tb = A2T.astype(bf16)
    A3Tb = A3T.astype(bf16)
    A1T = A2T = A3T = None

    # l1 is identical across a channel group (stage 1 is contraction-
    # sharded): [N, NTGT_TOT] with zero columns past nu
    l1_by_c = []
    for c in range(C):
        l1c = np.zeros((N, NTGT_TOT), bf16)
        l1c[:, :nu] = A1Tb[c][:, u]
        l1_by_c.append(l1c)

    in_maps = []
    for ci in range(NCORES):
        c, r = divmod(ci, NGRP)
        sl = slice(r * RLOC, (r + 1) * RLOC)
        in_maps.append({
            "l1": l1_by_c[c],
            "l2": A2Tb[c][sl, :],                       # row slab, contiguous
            "l3": np.ascontiguousarray(A3Tb[c][:, sl]),  # column slab
            "xw": xwb,
        })

    nc = _get_nc()
    _install_ntff_hook()
    trace = os.environ.get("GTN_TRACE", "1") != "0"
    t0 = time.time()
    res = None
    if trace:
        try:
            res = run_bass_kernel_spmd(nc, in_maps, list(range(NCORES)),
                                       trace=True,
                                       trace_cores=list(range(NCORES)))
        except Exception as e:
            import traceback
            traceback.print_exc()
            print(f"[kernel] trace run failed ({e!r}); retrying untraced")
            res = None
    if res is None:
        res = run_bass_kernel_spmd(nc, in_maps, list(range(NCORES)),
                                   trace=False)
    wall_ns = int((time.time() - t0) * 1e9)
    LAST_EXEC_NS = res.exec_time_ns if res.exec_time_ns else wall_ns
    LAST_RESULTS = res

    # core c*NGRP+r returns z[h, i, m*DOUT+d] = summed Zp for padded target
    # row  h*512 + m*128 + 32r + i  (SBUF-layout ReduceScatter blocks)
    NSPLIT = 2
    QT = NTGT_TOT // NSPLIT       # 512 padded target rows per split
    HM_ = QT // P                 # 4 m-tiles per split
    PB = P // NGRP                # 32 partitions per core block
    Zu = np.empty((C, nu, DOUT), np.float32)
    Zpad = np.empty((NTGT_TOT, DOUT), np.float32)
    for c in range(C):
        for r in range(NGRP):
            zc = np.asarray(res.results[c * NGRP + r]["z"], np.float32)
            blk = zc.reshape(NSPLIT, PB, HM_, DOUT).transpose(0, 2, 1, 3)
            for h in range(NSPLIT):
                for m in range(HM_):
                    row0 = QT * h + P * m + PB * r
                    Zpad[row0: row0 + PB] = blk[h, m]
        Zu[c] = Zpad[:nu]
    su = s[:, u]                                             # [C, nu]
    with np.errstate(divide="ignore", invalid="ignore"):
        sinv = np.where(su == 0, 0.0, 1.0 / su).astype(np.float32)
    Hn = Zu * sinv[:, :, None]                               # [C, nu, 128]
    Xc = np.maximum(Hn + np.asarray(gcn_b, np.float32)[None, None, :], 0.0)
    X_ = Xc.transpose(1, 0, 2).reshape(nu, C * W_OUT)        # [nu, 256]
    y = X_[inv] @ np.asarray(lin_w, np.float32)
    y = y + np.asarray(lin_b, np.float32)
    return y.astype(np.float32)
